# revision 1
# baseline (speedup 1.0000x reference)
"""AttentionPooling kernel for Trainium2 (8 NeuronCores, SPMD, no collectives).

reference math:
    scores = tanh(x @ W1 + b1) @ W2 + b2        # [N, 1]
    attn   = softmax(scores, axis=0)            # global over all N rows
    pooled = segment_sum(x * attn, batch, 1024) # [1024, 256]

Strategy:
  - batch is sorted, so shard ROWS at graph boundaries: core c gets all rows
    with batch in [128c, 128(c+1)).  Each core owns exactly 128 output graphs
    -> no cross-core reduction for pooled.
  - b2 cancels in softmax (constant shift) -> dropped.  b1 handled by an extra
    rank-1 matmul only if nonzero (it is zeros in the reference data).
  - softmax normalizer: each core returns unnormalized A_g = sum_i e_i x_i and
    per-graph e-sums; host divides by the global Z (exact).
  - per 128-row tile on device:
      xT   = transpose(x_tile) on PE            (PSUM->SBUF copy on DVE+ACT)
      hT   = W1^T xT   (f32r matmuls, N=256)
      thT  = tanh(hT)  on ACT (PSUM->SBUF fused)
      s    = thT^T W2  (N=1 matmuls -> PSUM)
      e    = exp(s)    on ACT
      M    = (iota == brel) * e   one fused DVE tensor_scalar
      acc[128g, 256] += M^T @ x_tile   (f32r, N=256, PSUM-resident accumulator)
      esum[128g, 1]  += M^T @ ones
"""

import numpy as np
from contextlib import ExitStack

import concourse.bass as bass
import concourse.bacc as bacc
import concourse.mybir as mybir
import concourse.tile as tile
from concourse.bass_utils import run_bass_kernel_spmd
from concourse.masks import make_identity

F32 = mybir.dt.float32
F32R = mybir.dt.float32r
I32 = mybir.dt.int32

NUM_GRAPHS = 1024
NC = 8
GPC = NUM_GRAPHS // NC  # graphs per core = 128
P = 128
D = 256
ST = 8  # tiles per DMA supertile (1 MiB chunks)
SG = 2  # tiles per score group


def build_program(R: int, T: int, with_b1: bool) -> bass.Bass:
    assert T % ST == 0 and R == T * P
    nsup = T // ST

    nc = bacc.Bacc("TRN2", target_bir_lowering=False, debug=False)
    xs = nc.declare_dram_parameter("xs", [R, D], F32R, isOutput=False)
    brel = nc.declare_dram_parameter("brel", [P, T], F32, isOutput=False)
    w1 = nc.declare_dram_parameter("w1", [D, D], F32, isOutput=False)
    w2 = nc.declare_dram_parameter("w2", [P, 2], F32, isOutput=False)
    if with_b1:
        b1d = nc.declare_dram_parameter("b1d", [1, D], F32, isOutput=False)
    pooled = nc.declare_dram_parameter("pooled", [P, D], F32, isOutput=True)
    evec_out = nc.declare_dram_parameter("evec_out", [P, T], F32, isOutput=True)

    with ExitStack() as ctx:
        tc = ctx.enter_context(tile.TileContext(nc))
        const = ctx.enter_context(tc.tile_pool(name="const", bufs=1))
        xpool = ctx.enter_context(tc.tile_pool(name="x", bufs=3))
        xtpp = ctx.enter_context(tc.tile_pool(name="xtp", bufs=3, space="PSUM"))
        xtsp = ctx.enter_context(tc.tile_pool(name="xts", bufs=2))
        htpp = ctx.enter_context(tc.tile_pool(name="htp", bufs=2, space="PSUM"))
        thp = ctx.enter_context(tc.tile_pool(name="th", bufs=2))
        spp = ctx.enter_context(tc.tile_pool(name="sp", bufs=2, space="PSUM"))
        epl = ctx.enter_context(tc.tile_pool(name="e", bufs=2))
        mpl = ctx.enter_context(tc.tile_pool(name="m", bufs=3))
        accp = ctx.enter_context(tc.tile_pool(name="acc", bufs=1, space="PSUM"))
        outp = ctx.enter_context(tc.tile_pool(name="out", bufs=1))

        # ---- constants ----
        identf = const.tile([P, P], F32, tag="identf")
        make_identity(nc, identf[:])
        ident = const.tile([P, P], F32R)
        nc.vector.tensor_copy(ident[:], identf[:])
        iota_i = const.tile([P, P], I32)
        nc.gpsimd.iota(iota_i[:], pattern=[[1, P]], base=0, channel_multiplier=0)
        iota_f = const.tile([P, P], F32)
        nc.vector.tensor_copy(iota_f[:], iota_i[:])

        w1f = const.tile([P, 2, D], F32, tag="w1f")  # [d_lo, dc, j]
        nc.sync.dma_start(w1f[:], w1.rearrange("(dc p) j -> p dc j", p=P))
        w1sb = const.tile([P, 2, D], F32R)
        nc.vector.tensor_copy(w1sb[:], w1f[:])
        w2f = const.tile([P, 2], F32, tag="w2f")  # [j_lo, jc]
        nc.sync.dma_start(w2f[:], w2[:])
        # fp32r matmuls need moving free-dim >= 2: duplicate W2 column
        w2r = []
        for jc in range(2):
            t = const.tile([P, 2], F32R, tag=f"w2r{jc}")
            nc.vector.tensor_copy(t[:], w2f[:, jc : jc + 1].to_broadcast([P, 2]))
            w2r.append(t)
        brelsb = const.tile([P, T], F32)
        nc.sync.dma_start(brelsb[:], brel[:])
        if with_b1:
            b1f = const.tile([1, D], F32, tag="b1f")  # [1, j]
            nc.sync.dma_start(b1f[:], b1d[:])
            b1sb = const.tile([1, D], F32R)
            nc.vector.tensor_copy(b1sb[:], b1f[:])
            ones_rf = const.tile([1, SG * P], F32, tag="ones_rf")
            nc.gpsimd.memset(ones_rf[:], 1.0)
            ones_row = const.tile([1, SG * P], F32R)
            nc.vector.tensor_copy(ones_row[:], ones_rf[:])

        evec = const.tile([P, T], F32, tag="evec")  # exp(s) per row
        # persistent PSUM accumulator
        acc = accp.tile([P, D], F32)  # pooled[g, d]

        Tanh = mybir.ActivationFunctionType.Tanh
        Exp = mybir.ActivationFunctionType.Exp

        for sup in range(nsup):
            xsb = xpool.tile([P, ST, D], F32R)
            src = xs[sup * ST * P : (sup + 1) * ST * P, :]
            nc.sync.dma_start(xsb[:], src.rearrange("(t p) d -> p t d", p=P))

            for g in range(ST // SG):
                # onehot for each tile: independent of exp -> build early
                ohs = []
                for tt in range(SG):
                    gt = sup * ST + g * SG + tt
                    oh = mpl.tile([P, P], F32, tag="oh")
                    nc.vector.tensor_scalar(
                        oh[:],
                        iota_f[:],
                        brelsb[:, gt : gt + 1],
                        None,
                        op0=mybir.AluOpType.is_equal,
                    )
                    ohs.append(oh)
                # transpose SG tiles: xtp[d_lo, dc, tt, i]
                xtp = xtpp.tile([P, 2, SG, P], F32R)
                for tt in range(SG):
                    t = g * SG + tt
                    for dc in range(2):
                        nc.tensor.transpose(
                            xtp[:, dc, tt, :],
                            xsb[:, t, dc * P : (dc + 1) * P],
                            ident[:],
                        )
                xts = xtsp.tile([P, 2, SG, P], F32R)
                nc.vector.tensor_copy(xts[:, 0], xtp[:, 0])
                nc.vector.tensor_copy(xts[:, 1, 0], xtp[:, 1, 0])
                nc.scalar.copy(xts[:, 1, 1], xtp[:, 1, 1])

                # hT[j_lo, jc, i] = sum_d W1[d, j] xT[d, i]
                htp = htpp.tile([P, 2, SG * P], F32)
                for jc in range(2):
                    for dc in range(2):
                        nc.tensor.matmul(
                            htp[:, jc, :],
                            lhsT=w1sb[:, dc, jc * P : (jc + 1) * P],
                            rhs=xts[:, dc],
                            start=(dc == 0),
                            stop=(dc == 1 and not with_b1),
                        )
                    if with_b1:
                        nc.tensor.matmul(
                            htp[:, jc, :],
                            lhsT=b1sb[:, jc * P : (jc + 1) * P],
                            rhs=ones_row[:],
                            start=False,
                            stop=True,
                        )
                th = thp.tile([P, 2, SG * P], F32R)
                nc.scalar.activation(th[:], htp[:], Tanh)

                # s[i] per tile -> PSUM columns
                sp = spp.tile([P, SG, 2], F32)
                for tt in range(SG):
                    for jc in range(2):
                        nc.tensor.matmul(
                            sp[:, tt, :],
                            lhsT=th[:, jc, tt * P : (tt + 1) * P],
                            rhs=w2r[jc][:],
                            start=(jc == 0),
                            stop=(jc == 1),
                            skip_group_check=True,
                        )
                gt0 = sup * ST + g * SG
                nc.scalar.activation(evec[:, gt0 : gt0 + SG], sp[:, :, 0], Exp)

                for tt in range(SG):
                    t = g * SG + tt
                    gt = sup * ST + t
                    m = mpl.tile([P, P], F32R)
                    nc.vector.tensor_scalar(
                        m[:],
                        ohs[tt][:],
                        evec[:, gt : gt + 1],
                        None,
                        op0=mybir.AluOpType.mult,
                    )
                    nc.tensor.matmul(
                        acc[:],
                        lhsT=m[:],
                        rhs=xsb[:, t, :],
                        start=(gt == 0),
                        stop=(gt == T - 1),
                        skip_group_check=True,
                    )

        out_sb = outp.tile([P, D], F32)
        nc.vector.tensor_copy(out_sb[:], acc[:])
        nc.sync.dma_start(pooled[:], out_sb[:])
        nc.sync.dma_start(evec_out[:], evec[:])

    nc.compile()
    return nc


def _prep_inputs(x, batch, W1, b1, W2):
    """Shard rows at graph boundaries; pad to a common multiple of ST*P rows."""
    x = np.ascontiguousarray(np.asarray(x, dtype=np.float32))
    batch = np.asarray(batch)
    bounds = np.searchsorted(batch, np.arange(0, NUM_GRAPHS + 1, GPC))
    counts = np.diff(bounds)
    chunk = ST * P
    R = int(np.ceil(max(int(counts.max()), 1) / chunk) * chunk)
    T = R // P

    w1h = np.ascontiguousarray(np.asarray(W1, dtype=np.float32))  # [d, j]
    w2h = np.ascontiguousarray(
        np.asarray(W2, dtype=np.float32).reshape(2, P).transpose(1, 0)
    )  # -> [j_lo, jc]
    b1h = np.asarray(b1, dtype=np.float32).reshape(1, D)
    with_b1 = bool(np.any(b1h))

    in_maps = []
    for c in range(NC):
        lo, hi = int(bounds[c]), int(bounds[c + 1])
        n = hi - lo
        xs = np.zeros((R, D), dtype=np.float32)
        xs[:n] = x[lo:hi]
        br = np.full((R,), -1.0, dtype=np.float32)
        br[:n] = (np.asarray(batch[lo:hi], dtype=np.int64) - c * GPC).astype(
            np.float32
        )
        brel_pt = np.ascontiguousarray(br.reshape(T, P).transpose(1, 0))  # [P, T]
        m = {"xs": xs, "brel": brel_pt, "w1": w1h, "w2": w2h}
        if with_b1:
            m["b1d"] = b1h
        in_maps.append(m)
    return in_maps, R, T, with_b1, [int(c) for c in counts]


def run(x, batch, W1, b1, W2, b2, trace=False, trace_kwargs=None):
    in_maps, R, T, with_b1, counts = _prep_inputs(x, batch, W1, b1, W2)
    nc = build_program(R, T, with_b1)
    res = run_bass_kernel_spmd(
        nc,
        in_maps,
        core_ids=list(range(NC)),
        trace=trace,
        **(trace_kwargs or {}),
    )
    A = np.concatenate(
        [res.results[c]["pooled"] for c in range(NC)], axis=0
    ).astype(np.float64)
    Z = 0.0
    for c in range(NC):
        ev = res.results[c]["evec_out"].astype(np.float64)  # [P, T]
        n = counts[c]
        rows = ev.transpose(1, 0).reshape(-1)  # row r = t*128+p order
        Z += rows[:n].sum()
    out = (A / Z).astype(np.float32)
    return out, res


def kernel(x, batch, W1, b1, W2, b2):
    out, _ = run(x, batch, W1, b1, W2, b2)
    return out



# revision 5
# speedup vs baseline: 1.7912x; 1.7912x over previous
"""AttentionPooling kernel for Trainium2 (8 NeuronCores, SPMD, no collectives).

reference math:
    scores = tanh(x @ W1 + b1) @ W2 + b2        # [N, 1]
    attn   = softmax(scores, axis=0)            # global over all N rows
    pooled = segment_sum(x * attn, batch, 1024) # [1024, 256]

Strategy (v2 — all-bf16 PE, no on-chip transposes):
  - batch is sorted, so shard ROWS at graph boundaries: core c gets all rows
    with batch in [128c, 128(c+1)).  Each core owns exactly 128 output graphs
    -> no cross-core reduction for pooled.
  - b2 cancels in softmax (constant shift) -> dropped.  b1 folded into the
    tanh activation as a per-partition bias (free).
  - host supplies TWO bf16 copies of x per core: row-major (pool-MM rhs) and
    transposed (score-GEMM rhs).  Same HBM bytes as one fp32 copy, and every
    matmul runs at full bf16 rate (fp32 PE mode is ~4x slower).
  - per 4-tile block (512 rows):
      hT[j, i] = sum_d W1[d, j] xT[d, i]   4 bf16 MMs, N=512 -> PSUM
      th       = tanh(hT + b1)             ACT, PSUM->SBUF bf16
      s        = th^T @ W2dup              th-as-stationary (FWL), N=2
      e        = exp(s)                    ACT -> evec[128, T] bf16
      m        = (iota == brel) * e        ONE fused DVE tensor_scalar, bf16
      acc[128g, 256] += m^T @ x_tile       bf16 MM, N=256, persistent PSUM
  - softmax normalizer: host divides the unnormalized per-core sums by the
    global Z computed from the returned evec (exact, float64).
  - emission is software-pipelined 3 block-groups deep so PE never waits on
    ACT/DVE, and supertile (16-tile) DMAs prefetch one supertile ahead.
"""

import numpy as np
import ml_dtypes
from contextlib import ExitStack

import concourse.bass as bass
import concourse.bacc as bacc
import concourse.mybir as mybir
import concourse.tile as tile
from concourse.bass_utils import run_bass_kernel_spmd

F32 = mybir.dt.float32
BF16 = mybir.dt.bfloat16
I32 = mybir.dt.int32
BF = ml_dtypes.bfloat16

NUM_GRAPHS = 1024
NC = 8
GPC = NUM_GRAPHS // NC  # graphs per core = 128
P = 128
D = 256
ST = 16         # tiles per DMA supertile (2048 rows)
BLK = 4         # tiles per hT/score block (512 rows)


def build_program(R: int, T: int, with_b1: bool) -> bass.Bass:
    assert R == T * P and T % ST == 0
    nsup = T // ST
    nblk = T // BLK
    bps = ST // BLK  # blocks per supertile

    nc = bacc.Bacc("TRN2", target_bir_lowering=False, debug=False)
    xs_d = nc.declare_dram_parameter("xs", [nsup * P, ST * D], BF16, isOutput=False)
    xst_d = nc.declare_dram_parameter("xst", [nsup * P, 2 * ST * P], BF16, isOutput=False)
    w1_d = nc.declare_dram_parameter("w1", [P, 2 * 2 * P], BF16, isOutput=False)
    w2_d = nc.declare_dram_parameter("w2", [P, 2 * 2], BF16, isOutput=False)
    brel_d = nc.declare_dram_parameter("brel", [P, T], F32, isOutput=False)
    if with_b1:
        b1_d = nc.declare_dram_parameter("b1d", [P, 2], F32, isOutput=False)
    pooled = nc.declare_dram_parameter("pooled", [P, D], F32, isOutput=True)
    evec_out = nc.declare_dram_parameter("evec_out", [P, T], F32, isOutput=True)

    Tanh = mybir.ActivationFunctionType.Tanh
    Exp = mybir.ActivationFunctionType.Exp

    with ExitStack() as ctx:
        tc = ctx.enter_context(tile.TileContext(nc))
        const = ctx.enter_context(tc.tile_pool(name="const", bufs=1))
        xsp = ctx.enter_context(tc.tile_pool(name="xs", bufs=3))
        xstp = ctx.enter_context(tc.tile_pool(name="xst", bufs=2))
        htpp = ctx.enter_context(tc.tile_pool(name="htp", bufs=2, space="PSUM"))
        thp = ctx.enter_context(tc.tile_pool(name="th", bufs=3))
        spp = ctx.enter_context(tc.tile_pool(name="sp", bufs=2, space="PSUM"))
        mp = ctx.enter_context(tc.tile_pool(name="m", bufs=3))
        accp = ctx.enter_context(tc.tile_pool(name="acc", bufs=1, space="PSUM"))
        outp = ctx.enter_context(tc.tile_pool(name="out", bufs=1))

        # ---- constants ----
        iota_i = const.tile([P, P], I32)
        nc.gpsimd.iota(iota_i[:], pattern=[[1, P]], base=0, channel_multiplier=0)
        iota_bf = const.tile([P, P], BF16)
        nc.vector.tensor_copy(iota_bf[:], iota_i[:])

        w1sb = const.tile([P, 2, 2, P], BF16, tag="w1sb")  # [d_lo, dc, jc, j_lo]
        nc.sync.dma_start(w1sb[:], w1_d.rearrange("p (a b j) -> p a b j", a=2, b=2))
        w2sb = const.tile([P, 2, 2], BF16, tag="w2sb")  # [j_lo, jc, dup]
        nc.sync.dma_start(w2sb[:], w2_d.rearrange("p (a r) -> p a r", a=2))
        brelsb = const.tile([P, T], F32, tag="brelsb")
        nc.sync.dma_start(brelsb[:], brel_d[:])
        if with_b1:
            b1sb = const.tile([P, 2], F32, tag="b1sb")  # [j_lo, jc]
            nc.sync.dma_start(b1sb[:], b1_d[:])

        evec = const.tile([P, T], F32, tag="evec")  # exp(s) per row
        acc = accp.tile([P, D], F32)  # pooled[g, d], persistent PSUM bank

        xs_t = {}
        xst_t = {}
        htp_t = {}
        th_t = {}
        sp_t = {}
        m_t = {}

        def issue_sup_dma(s):
            xs_t[s] = xsp.tile([P, ST, D], BF16, tag="xsb", name=f"xsb{s}")
            nc.sync.dma_start(
                xs_t[s][:],
                xs_d[s * P : (s + 1) * P, :].rearrange("p (t d) -> p t d", t=ST),
            )
            xst_t[s] = xstp.tile([P, 2, ST * P], BF16, tag="xstb", name=f"xstb{s}")
            nc.sync.dma_start(
                xst_t[s][:],
                xst_d[s * P : (s + 1) * P, :].rearrange("p (c i) -> p c i", c=2),
            )

        issue_sup_dma(0)

        for g in range(nblk + 3):
            # ---- stage A: DMA prefetch + hT GEMM for block g ----
            if g < nblk:
                s, bis = divmod(g, bps)
                if bis == 0 and s + 1 < nsup:
                    issue_sup_dma(s + 1)
                htp_t[g] = htpp.tile([P, 2, BLK * P], F32, tag="htp", name=f"htp{g}")
                for jc in range(2):
                    for dc in range(2):
                        nc.tensor.matmul(
                            htp_t[g][:, jc, :],
                            lhsT=w1sb[:, dc, jc, :],
                            rhs=xst_t[s][:, dc, bis * BLK * P : (bis + 1) * BLK * P],
                            start=(dc == 0),
                            stop=(dc == 1),
                        )

            # ---- stage B: tanh for block g-1 ----
            b = g - 1
            if 0 <= b < nblk:
                th_t[b] = thp.tile([P, 2, BLK * P], BF16, tag="th", name=f"th{b}")
                for jc in range(2):
                    nc.scalar.activation(
                        th_t[b][:, jc],
                        htp_t[b][:, jc],
                        Tanh,
                        bias=(b1sb[:, jc : jc + 1] if with_b1 else 0.0),
                    )
                del htp_t[b]

            # ---- stage C: scores + exp + masks for block g-2 ----
            b = g - 2
            if 0 <= b < nblk:
                sp_t[b] = spp.tile([P, BLK, 2], F32, tag="sp", name=f"sp{b}")
                for tt in range(BLK):
                    for jc in range(2):
                        nc.tensor.matmul(
                            sp_t[b][:, tt, :],
                            lhsT=th_t[b][:, jc, tt * P : (tt + 1) * P],
                            rhs=w2sb[:, jc, :],
                            start=(jc == 0),
                            stop=(jc == 1),
                            skip_group_check=True,
                        )
                c0 = b * BLK
                nc.scalar.activation(evec[:, c0 : c0 + BLK], sp_t[b][:, :, 0], Exp)
                m_t[b] = mp.tile([P, BLK, P], BF16, tag="m", name=f"m{b}")
                for tt in range(BLK):
                    col = c0 + tt
                    nc.vector.tensor_scalar(
                        m_t[b][:, tt, :],
                        iota_bf[:],
                        brelsb[:, col : col + 1],
                        evec[:, col : col + 1],
                        op0=mybir.AluOpType.is_equal,
                        op1=mybir.AluOpType.mult,
                    )
                del th_t[b], sp_t[b]

            # ---- stage D: pooling matmuls for block g-3 ----
            b = g - 3
            if 0 <= b < nblk:
                s, bis = divmod(b, bps)
                for tt in range(BLK):
                    t_glob = b * BLK + tt
                    nc.tensor.matmul(
                        acc[:],
                        lhsT=m_t[b][:, tt, :],
                        rhs=xs_t[s][:, bis * BLK + tt, :],
                        start=(t_glob == 0),
                        stop=(t_glob == T - 1),
                        skip_group_check=True,
                    )
                del m_t[b]

        out_sb = outp.tile([P, D], F32)
        nc.vector.tensor_copy(out_sb[:], acc[:])
        nc.sync.dma_start(pooled[:], out_sb[:])
        nc.sync.dma_start(evec_out[:], evec[:])

    nc.compile()
    return nc


def _prep_inputs(x, batch, W1, b1, W2):
    """Shard rows at graph boundaries; build bf16 supertile layouts."""
    x16 = np.asarray(x, dtype=np.float32).astype(BF)
    batch = np.asarray(batch)
    bounds = np.searchsorted(batch, np.arange(0, NUM_GRAPHS + 1, GPC))
    counts = np.diff(bounds)
    chunk = ST * P
    R = int(np.ceil(max(int(counts.max()), 1) / chunk) * chunk)
    T = R // P
    nsup = T // ST

    b1h = np.asarray(b1, dtype=np.float32).reshape(-1)
    with_b1 = bool(np.any(b1h))
    # [d_lo, dc, jc, j_lo]
    w1h = np.ascontiguousarray(
        np.asarray(W1, dtype=np.float32)
        .reshape(2, P, 2, P)
        .transpose(1, 0, 2, 3)
    ).astype(BF).reshape(P, 2 * 2 * P)
    # [j_lo, jc, dup2]
    w2h = np.repeat(
        np.asarray(W2, dtype=np.float32).reshape(2, P).transpose(1, 0)[:, :, None],
        2,
        axis=2,
    ).astype(BF).reshape(P, 4)
    b1_pt = np.ascontiguousarray(
        b1h.reshape(2, P).transpose(1, 0)
    )  # [j_lo, jc] f32

    in_maps = []
    for c in range(NC):
        lo, hi = int(bounds[c]), int(bounds[c + 1])
        n = hi - lo
        xs = np.zeros((R, D), dtype=BF)
        xs[:n] = x16[lo:hi]
        # [s, p, t, d] <- row s*2048 + t*128 + p
        xs_h = np.ascontiguousarray(
            xs.reshape(nsup, ST, P, D).transpose(0, 2, 1, 3)
        ).reshape(nsup * P, ST * D)
        # [s, d_lo, dc, i] <- x[s*2048 + i, dc*128 + d_lo]
        xst_h = np.ascontiguousarray(
            xs.reshape(nsup, ST * P, 2, P).transpose(0, 3, 2, 1)
        ).reshape(nsup * P, 2 * ST * P)
        br = np.full((R,), -1.0, dtype=np.float32)
        br[:n] = (np.asarray(batch[lo:hi], dtype=np.int64) - c * GPC).astype(
            np.float32
        )
        brel_pt = np.ascontiguousarray(br.reshape(T, P).transpose(1, 0))  # [P, T]
        m = {"xs": xs_h, "xst": xst_h, "w1": w1h, "w2": w2h, "brel": brel_pt}
        if with_b1:
            m["b1d"] = b1_pt
        in_maps.append(m)
    return in_maps, R, T, with_b1, [int(c) for c in counts]


def run(x, batch, W1, b1, W2, b2, trace=False, trace_kwargs=None):
    in_maps, R, T, with_b1, counts = _prep_inputs(x, batch, W1, b1, W2)
    nc = build_program(R, T, with_b1)
    res = run_bass_kernel_spmd(
        nc,
        in_maps,
        core_ids=list(range(NC)),
        trace=trace,
        **(trace_kwargs or {}),
    )
    A = np.concatenate(
        [res.results[c]["pooled"] for c in range(NC)], axis=0
    ).astype(np.float64)
    Z = 0.0
    for c in range(NC):
        # device pools with bf16-rounded e; round here too so Z matches
        ev = res.results[c]["evec_out"].astype(BF).astype(np.float64)  # [P, T]
        n = counts[c]
        rows = ev.transpose(1, 0).reshape(-1)  # row r = t*128+p order
        Z += rows[:n].sum()
    out = (A / Z).astype(np.float32)
    return out, res


def kernel(x, batch, W1, b1, W2, b2):
    out, _ = run(x, batch, W1, b1, W2, b2)
    return out
